# revision 13
# baseline (speedup 1.0000x reference)
"""Causal multi-head attention (B=4, H=16, S=2048, D=64) on 8 TRN2 NeuronCores.

Sharding: the 64 (batch, head) pairs are fully independent; each core gets 8
pairs. q/k are pre-transposed to d-major [64, 2048] and converted to bf16 on
the host during sharding, so every DMA is contiguous and the TensorEngine
runs single-pass bf16 matmuls (fp32 matmuls cost two PE passes).

Per-core algorithm (flash-attention, transposed-score layout), per pair and
per q-tile of 512 columns ("unit"):
  for each k-tile of 128 rows (causal: only k-tiles intersecting the lower
  triangle; diagonal blocks at reduced width), in groups of 2:
    S^T[k,q] = matmul(lhsT=K^T tile [64,128], rhs=Q^T tile [64,w])  (PSUM)
    P = exp(S^T * 1/8) -> bf16 SBUF             (ScalarE, PSUM->SBUF)
    diagonal blocks: zero the masked (q<k) triangle (GPSIMD affine_select)
    acc[65,512] += matmul(lhsT=V'[128,65], rhs=P)   V' has a ones column,
      so acc row 64 accumulates the softmax denominator for free.
  tail: evict acc -> SBUF bf16 [65,512]; per 128-col q-block:
    tp[128,65] = matmul(lhsT=osb[65,128], rhs=I65)  (transpose via matmul:
      tp cols 0..63 = out[q,d] un-normalized, col 64 = denominator)
    rcol = 1/tp[:,64]; out = tp[:,0:64] * rcol      (per-partition scalar)
    DMA out[q-block, 64] fp32 to DRAM (contiguous)

The PE queue executes in order, so everything that waits on another engine
is emitted lagged: PV matmuls trail their S^T group by 2 groups (the exp on
ScalarE is the pacing engine), and each unit's tail is emitted after the
next unit's first S^T group.  This keeps the PE stream dense (HAM stays at
full clock) and hides ScalarE/VectorE latency.

Output per core is [8*2048, 64] in natural [q, d] layout; the host only
scatters slices back into the full [4, 2048, 1024] array.
"""

import math

import numpy as np
import ml_dtypes

import concourse.bass as bass
import concourse.bacc as bacc
import concourse.tile as tile
import concourse.mybir as mybir
from concourse import bass_utils
from concourse.masks import make_identity

B, H, S, D = 4, 16, 2048, 64
N_CORES = 8
PAIRS = (B * H) // N_CORES  # 8 pairs per core
QT = 512                    # q-tile width
KT = 128                    # k-tile rows (PE contraction tile)
NQT = S // QT               # 4 q-tiles per pair
SCALE = 1.0 / math.sqrt(D)
PV_LAG = 1                  # PV matmul group lag behind S^T groups
BF16 = ml_dtypes.bfloat16

_COMPILED = {}


def build_nc():
    nc = bacc.Bacc(
        "TRN2",
        target_bir_lowering=False,
        debug=False,
        enable_asserts=True,
        num_devices=N_CORES,
    )
    f32 = mybir.dt.float32
    bf16 = mybir.dt.bfloat16

    qt_d = nc.dram_tensor("qt", [PAIRS * D, S], bf16, kind="ExternalInput").ap()
    kt_d = nc.dram_tensor("kt", [PAIRS * D, S], bf16, kind="ExternalInput").ap()
    v_d = nc.dram_tensor("v", [PAIRS * S, D], bf16, kind="ExternalInput").ap()
    out_d = nc.dram_tensor("out", [PAIRS * S, D], f32, kind="ExternalOutput").ap()

    with tile.TileContext(nc) as tc:
        with (
            tc.tile_pool(name="consts", bufs=1) as consts,
            tc.tile_pool(name="qk", bufs=2) as qk_pool,
            tc.tile_pool(name="vp", bufs=2) as v_pool,
            tc.tile_pool(name="pp", bufs=10) as p_pool,
            tc.tile_pool(name="op", bufs=2) as o_pool,
            tc.tile_pool(name="fp", bufs=4) as f_pool,
            tc.tile_pool(name="rp", bufs=4) as r_pool,
            tc.tile_pool(name="ps", bufs=4, space="PSUM") as ps_pool,
            tc.tile_pool(name="acc", bufs=2, space="PSUM") as acc_pool,
            tc.tile_pool(name="tp", bufs=2, space="PSUM") as tp_pool,
        ):
            # I65: 65x65 identity for the transpose-matmul.
            ident = consts.tile([D + 1, D + 1], bf16)
            make_identity(nc, ident)

            sbufs = {}

            def load_pair(p):
                qt_sb = qk_pool.tile([D, S], bf16, tag="qt")
                kt_sb = qk_pool.tile([D, S], bf16, tag="kt")
                nc.sync.dma_start(out=qt_sb, in_=qt_d[p * D:(p + 1) * D, :])
                nc.sync.dma_start(out=kt_sb, in_=kt_d[p * D:(p + 1) * D, :])
                v_sb = v_pool.tile([KT, S // KT, D + 1], bf16)
                nc.gpsimd.memset(v_sb[:, :, D:D + 1], 1.0)
                nc.sync.dma_start(
                    out=v_sb[:, :, 0:D],
                    in_=v_d[p * S:(p + 1) * S, :].rearrange(
                        "(t kp) d -> kp t d", kp=KT),
                )
                sbufs[p] = (qt_sb, kt_sb, v_sb)

            def emit_st_group(p, j, g):
                qt_sb, kt_sb, _ = sbufs[p]
                tiles = []
                for half in range(2):
                    t = 2 * g + half
                    off = max(0, KT * t - QT * j)
                    w = QT - off
                    ps = ps_pool.tile([KT, QT], f32)
                    nc.tensor.matmul(
                        ps[:, 0:w],
                        lhsT=kt_sb[:, KT * t:KT * (t + 1)],
                        rhs=qt_sb[:, QT * j + off:QT * (j + 1)],
                        start=True, stop=True,
                    )
                    p_sb = p_pool.tile([KT, QT], bf16)
                    nc.scalar.activation(
                        out=p_sb[:, 0:w], in_=ps[:, 0:w],
                        func=mybir.ActivationFunctionType.Exp,
                        scale=SCALE,
                    )
                    if t >= (QT // KT) * j:  # diagonal block: zero q < k
                        nc.gpsimd.affine_select(
                            out=p_sb[:, 0:w], in_=p_sb[:, 0:w],
                            compare_op=mybir.AluOpType.is_ge,
                            fill=0.0, base=0,
                            pattern=[[1, w]], channel_multiplier=-1,
                        )
                    tiles.append((p_sb, off))
                return tiles

            def emit_pv_group(acc, nkt, g, tiles):
                _, _, v_sb = sbufs[cur_pair[0]]
                for half in range(2):
                    t = 2 * g + half
                    p_sb, off = tiles[half]
                    nc.tensor.matmul(
                        acc[:, off:QT],
                        lhsT=v_sb[:, t, :],
                        rhs=p_sb[:, 0:QT - off],
                        start=(t == 0), stop=(t == nkt - 1),
                    )

            def emit_tail(p, j, acc):
                osb = o_pool.tile([D + 1, QT], bf16)
                nc.vector.tensor_copy(osb, acc)
                for b in range(QT // KT):
                    tp = tp_pool.tile([KT, D + 1], f32)
                    nc.tensor.matmul(
                        tp,
                        lhsT=osb[:, KT * b:KT * (b + 1)],
                        rhs=ident,
                        start=True, stop=True,
                    )
                    rcol = r_pool.tile([KT, 1], f32)
                    nc.vector.reciprocal(rcol, tp[:, D:D + 1])
                    fsb = f_pool.tile([KT, D], f32)
                    nc.vector.tensor_scalar_mul(fsb, tp[:, 0:D], rcol)
                    row0 = p * S + QT * j + KT * b
                    nc.sync.dma_start(out=out_d[row0:row0 + KT, :], in_=fsb)

            pending_tail = None  # (p, j, acc) whose tail not yet emitted
            cur_pair = [0]
            for p in range(PAIRS):
                cur_pair[0] = p
                load_pair(p)
                for j in range(NQT):
                    acc = acc_pool.tile([D + 1, QT], f32)
                    nkt = (QT // KT) * (j + 1)
                    ngr = nkt // 2
                    pend = []
                    for g in range(ngr):
                        pend.append((g, emit_st_group(p, j, g)))
                        if g == 0 and pending_tail is not None:
                            emit_tail(*pending_tail)
                            pending_tail = None
                        if len(pend) > PV_LAG:
                            gg, tiles = pend.pop(0)
                            emit_pv_group(acc, nkt, gg, tiles)
                    for gg, tiles in pend:
                        emit_pv_group(acc, nkt, gg, tiles)
                    pending_tail = (p, j, acc)
            emit_tail(*pending_tail)

    nc.compile()
    return nc


def _get_nc():
    if "nc" not in _COMPILED:
        _COMPILED["nc"] = build_nc()
    return _COMPILED["nc"]


def make_in_maps(q, k, v):
    q = np.asarray(q, dtype=np.float32).reshape(B * H, S, D)
    k = np.asarray(k, dtype=np.float32).reshape(B * H, S, D)
    v = np.asarray(v, dtype=np.float32).reshape(B * H, S, D)
    in_maps = []
    for c in range(N_CORES):
        sl = slice(c * PAIRS, (c + 1) * PAIRS)
        in_maps.append({
            "qt": np.ascontiguousarray(
                q[sl].transpose(0, 2, 1)).reshape(PAIRS * D, S).astype(BF16),
            "kt": np.ascontiguousarray(
                k[sl].transpose(0, 2, 1)).reshape(PAIRS * D, S).astype(BF16),
            "v": np.ascontiguousarray(v[sl]).reshape(PAIRS * S, D).astype(BF16),
        })
    return in_maps


def assemble(results):
    out = np.empty((B * H, S, D), dtype=np.float32)
    for c in range(N_CORES):
        out[c * PAIRS:(c + 1) * PAIRS] = results[c]["out"].reshape(PAIRS, S, D)
    return np.ascontiguousarray(
        out.reshape(B, H, S, D).transpose(0, 2, 1, 3).reshape(B, S, H * D))


def kernel(q, k, v):
    nc = _get_nc()
    res = bass_utils.run_bass_kernel_spmd(
        nc, make_in_maps(q, k, v), core_ids=list(range(N_CORES)))
    return assemble(res.results)
